# revision 25
# baseline (speedup 1.0000x reference)
"""ChaosSSMCore (diag mode) Trainium2 kernel.

Problem: B=4, S=4096, D=1024, fp32.
    delta  = softplus(x @ Wd.T); decay = exp(-delta * sigmoid(log_a))
    update = sigmoid(x @ Ws.T) * tanh(x @ Wi.T)
    gate   = sigmoid(x @ Wg.T)
    state_t = decay_t * state_{t-1} + update_t        (scan over S, elementwise in D)
    y      = (gate * states) @ Wo.T

Sharding: 8 cores = 4 batches x 2 halves of the D output dim. Each core
computes its 512-channel slice of the 4 input projections in [e, s] layout
(channels on partitions, sequence on the free axis), applies activations on
the scalar engine, runs the hardware tensor_tensor_scan (state = d*s + u along
the free dim) on the vector engine, gates, and computes a partial output GEMM
against its 512 columns of W_out. Host sums the two partials per batch and
transposes back.

ACT table sets on trn2 don't contain Softplus, and Exp/Ln live in a different
set than Tanh, so:
  softplus(z) = ln(exp(z) + 1)              (Exp + Ln, one table set)
  sigmoid(z)  = (1 + tanh(z/2)) / 2         (Tanh set; the 1/2 input scale is
                                             folded into W_select/W_gate and
                                             the two 1/2 output scales into
                                             W_out, which becomes W_out/4)
Per sequence chunk all Exp/Ln work runs before all Tanh work, and the ACT
instruction order is pinned with explicit dep edges so the scheduler cannot
interleave phases across chunks: 2 table loads per chunk.

Matmuls run in float32r (full-rate 4-byte matmul path).
"""

import sys

if "/opt/trn_rl_repo" not in sys.path:
    sys.path.insert(0, "/opt/trn_rl_repo")

import numpy as np

# Problem constants (hardcoded per harness contract).
B, S, D = 4, 4096, 1024
P = 128           # SBUF partitions
E = D // 2        # per-core channel slice
NC = 512          # sequence chunk (= fp32 moving-operand max = one PSUM bank)
KT = D // P       # k-tiles per input projection contraction (8)
ET = E // P       # e-tiles per core (4)
KO = E // P       # k'-tiles for the output GEMM contraction (4)
MT = D // P       # output-row tiles (8)
SC = S // NC      # sequence chunks (8)
N_CORES = 8

_CACHE = {}


def _build_program():
    import concourse.bacc as bacc
    import concourse.mybir as mybir
    import concourse.tile as tile
    from concourse.tile import add_dep_helper
    from contextlib import ExitStack

    f32 = mybir.dt.float32
    f32r = mybir.dt.float32r
    AF = mybir.ActivationFunctionType
    OP = mybir.AluOpType

    # The act-table-load inserter picks the FIRST table set containing each
    # activation func. Ln's first set (natural_log) lacks Exp and Exp's
    # (exp_and_friends after natural_log_exp) lacks Ln, so an Exp-Ln-Exp
    # chain reloads tables on every instruction (~2.7us each). Removing the
    # two funcs from their single-func sets (set membership only — dict
    # order, and therefore act_func_set_id numbering, is unchanged) makes
    # both resolve to natural_log_exp_and_others.
    from concourse.hw_specs import get_activation_tables

    tabs = get_activation_tables("gen3")
    tabs["natural_log"].discard(mybir.ActivationFunctionType.Ln)
    tabs["exp_and_friends"].discard(mybir.ActivationFunctionType.Exp)

    nc = bacc.Bacc("TRN2", target_bir_lowering=False)

    xt = nc.declare_dram_parameter("xt", [D, S], f32r, isOutput=False)
    w4 = nc.declare_dram_parameter("w4", [4, D, E], f32r, isOutput=False)
    wo = nc.declare_dram_parameter("wo", [E, D], f32r, isOutput=False)
    na = nc.declare_dram_parameter("na", [P, ET], f32, isOutput=False)
    yt = nc.declare_dram_parameter("yt", [D, S], f32, isOutput=True)

    xt_p = xt.rearrange("(k p) s -> p k s", p=P)        # [P, KT, S]
    w4_p = w4.rearrange("q (k p) e -> q p k e", p=P)    # [4, P, KT, E]
    wo_p = wo.rearrange("(k p) d -> p k d", p=P)        # [P, KO, D]
    yt_m = yt.rearrange("(m p) s -> m p s", p=P)        # [MT, P, S]

    with tile.TileContext(nc) as tc, ExitStack() as ctx:
        wpool = ctx.enter_context(tc.tile_pool(name="w", bufs=1))
        xpool = ctx.enter_context(tc.tile_pool(name="x", bufs=2))
        ppd = ctx.enter_context(tc.tile_pool(name="ppd", bufs=3, space="PSUM"))
        pp = ctx.enter_context(tc.tile_pool(name="pp", bufs=3, space="PSUM"))
        pyp = ctx.enter_context(tc.tile_pool(name="pyp", bufs=2, space="PSUM"))
        dpool = ctx.enter_context(tc.tile_pool(name="dp", bufs=2))
        decpool = ctx.enter_context(tc.tile_pool(name="dec", bufs=6))
        spool = ctx.enter_context(tc.tile_pool(name="sp", bufs=4))
        tpool = ctx.enter_context(tc.tile_pool(name="tp", bufs=4))
        upool = ctx.enter_context(tc.tile_pool(name="up", bufs=2))
        stpool = ctx.enter_context(tc.tile_pool(name="stp", bufs=6))
        gpool = ctx.enter_context(tc.tile_pool(name="gp", bufs=4))
        gdpool = ctx.enter_context(tc.tile_pool(name="gdp", bufs=6))
        ypool = ctx.enter_context(tc.tile_pool(name="yp", bufs=2))

        # Pin the ACT instruction order to the emission order so the
        # scheduler can't interleave Exp/Ln and Tanh phases (each crossing
        # costs a ~2.7us ACT table load).
        last_act = [None]

        def act(*args, **kwargs):
            h = nc.scalar.activation(*args, **kwargs)
            if last_act[0] is not None:
                add_dep_helper(h.ins, last_act[0].ins, sync=False,
                               reason="pin ACT table phase order")
            last_act[0] = h
            return h

        def load_w(q, split=False):
            wq = wpool.tile([P, KT * E], f32r, name=f"w{q}_sb", tag=f"w{q}")
            if split:
                # Per-k-tile DMAs: the first matmul only waits for its own
                # k-slice instead of the full 2MB tensor.
                for k in range(KT):
                    nc.sync.dma_start(
                        wq[:, k * E:(k + 1) * E], w4_p[q, :, k, :]
                    )
            else:
                nc.sync.dma_start(wq.rearrange("p (k e) -> p k e", k=KT), w4_p[q])
            return wq

        def load_x(c):
            x_sb = xpool.tile([P, KT * NC], f32r, name="x_sb", tag="x")
            for k in range(KT):
                nc.sync.dma_start(
                    x_sb[:, k * NC:(k + 1) * NC],
                    xt_p[:, k, c * NC:(c + 1) * NC],
                )
            return x_sb

        # Startup order: interleave the delta-weight and first-x k-slices so
        # the first matmul starts after ~2 small DMAs, then stream the three
        # gate weights. wo/na are not needed until the first emit_back
        # (~90us in) and load last.
        w_sb = [None] * 4
        w_sb[0] = wpool.tile([P, KT * E], f32r, name="w0_sb", tag="w0")
        x_next = xpool.tile([P, KT * NC], f32r, name="x_sb", tag="x")
        na_sb = wpool.tile([P, ET], f32, name="na_sb", tag="na")
        # The very first matmul group (phase A, j=0) only needs the j0
        # column-slice of each w0 k-tile; stream those 64KB slices (with the
        # x k-slices) first so the PE starts ~as early as possible.
        for k in range(KT):
            nc.sync.dma_start(
                w_sb[0][:, k * E:k * E + P], w4_p[0, :, k, 0:P]
            )
            nc.sync.dma_start(
                x_next[:, k * NC:(k + 1) * NC], xt_p[:, k, 0:NC]
            )
        for k in range(KT):
            nc.sync.dma_start(
                w_sb[0][:, k * E + P:(k + 1) * E], w4_p[0, :, k, P:E]
            )
        nc.sync.dma_start(na_sb[:, :], na[:, :])
        for q in (1, 2, 3):
            w_sb[q] = load_w(q, split=True)
        wo_sb = wpool.tile([P, KO * D], f32r, name="wo_sb", tag="wo")
        nc.sync.dma_start(wo_sb.rearrange("p (k d) -> p k d", k=KO), wo_p)

        prev_states = [None] * ET
        gated_tiles = [[None] * ET for _ in range(SC)]

        def mm_group(ps, q, j, x_sb):
            for k in range(KT):
                nc.tensor.matmul(
                    ps[:, :],
                    w_sb[q][:, k * E + j * P: k * E + (j + 1) * P],
                    x_sb[:, k * NC:(k + 1) * NC],
                    start=(k == 0),
                    stop=(k == KT - 1),
                )

        def phase_a(c, x_sb):
            # Decay path (Exp/Ln table). All four Exps run first so every
            # delta psum is drained before the Ln table-load (~2.7us) stalls
            # the ACT engine; decay lands in SBUF.
            dec_t = []
            e1_t = []
            for j in range(ET):
                ps = ppd.tile([P, NC], f32, name="psd", tag="ppd")
                mm_group(ps, 0, j, x_sb)
                e1 = dpool.tile([P, NC], f32, name="e1", tag="e1", bufs=5)
                act(e1[:, :], ps[:, :], AF.Exp)
                e1_t.append(e1)
            for j in range(ET):
                l1 = dpool.tile([P, NC], f32, name="l1", tag="l1")
                act(l1[:, :], e1_t[j][:, :], AF.Ln, bias=1.0)
                d = decpool.tile([P, NC], f32, name="dec", tag="dec")
                act(d[:, :], l1[:, :], AF.Exp, scale=na_sb[:, j:j + 1])
                dec_t.append(d)
            return dec_t

        def phase_b(c, x_sb):
            # The three gate projections (Tanh table).
            tS, tI, tG = [], [], []
            for j in range(ET):
                for q, lst, pool, nm in (
                    (1, tS, spool, "tS"),
                    (2, tI, tpool, "tI"),
                    (3, tG, gpool, "tG"),
                ):
                    ps = pp.tile([P, NC], f32, name="ps", tag="pp")
                    mm_group(ps, q, j, x_sb)
                    t = pool.tile([P, NC], f32, name=nm, tag=nm)
                    act(t[:, :], ps[:, :], AF.Tanh)
                    lst.append(t)
            return tS, tI, tG

        def emit_front(c, x_sb, ba_order=False):
            # For the last chunk run phase B before phase A: the Tanh work
            # overlaps the remaining PE matmuls, the final scan chain starts
            # earlier, and B(last) reuses B(prev)'s table (one load saved).
            if ba_order:
                tS, tI, tG = phase_b(c, x_sb)
                dec_t = phase_a(c, x_sb)
            else:
                dec_t = phase_a(c, x_sb)
                tS, tI, tG = phase_b(c, x_sb)

            # Phase C: update' = (1+tS)*tI ; scan ; gated' = (1+tG)*st.
            for j in range(ET):
                u = upool.tile([P, NC], f32, name="upd", tag="upd")
                nc.vector.scalar_tensor_tensor(
                    u[:, :], tS[j][:, :], 1.0, tI[j][:, :],
                    op0=OP.add, op1=OP.mult,
                )
                st = stpool.tile([P, NC], f32, name="st", tag="st")
                init = 0.0 if c == 0 else prev_states[j][:, NC - 1:NC]
                nc.vector.tensor_tensor_scan(
                    st[:, :], dec_t[j][:, :], u[:, :], init,
                    op0=OP.mult, op1=OP.add,
                )
                prev_states[j] = st
                g = gdpool.tile([P, NC], f32r, name="gated", tag="gated")
                nc.vector.scalar_tensor_tensor(
                    g[:, :], tG[j][:, :], 1.0, st[:, :],
                    op0=OP.add, op1=OP.mult,
                )
                gated_tiles[c][j] = g

        def emit_back(c):
            csl = slice(c * NC, (c + 1) * NC)
            for m in range(MT):
                py = pyp.tile([P, NC], f32, name="py", tag="py")
                for j in range(KO):
                    nc.tensor.matmul(
                        py[:, :],
                        wo_sb[:, j * D + m * P: j * D + (m + 1) * P],
                        gated_tiles[c][j][:, :],
                        start=(j == 0),
                        stop=(j == KO - 1),
                    )
                y_sb = ypool.tile([P, NC], f32, name="y_sb", tag="ysb")
                nc.vector.tensor_copy(y_sb[:, :], py[:, :])
                nc.sync.dma_start(yt_m[m, :, csl], y_sb[:, :])
            gated_tiles[c] = [None] * KO

        def emit_back_final(c):
            # Last chunk: no more projection matmuls will run, so all 8 PSUM
            # banks are free. Accumulate j-major into 8 live psum tiles so
            # the PE streams 8 matmuls per gated tile as soon as it lands
            # instead of stalling on each j four times.
            csl = slice(c * NC, (c + 1) * NC)
            pools = [
                (pyp, "py"), (pyp, "py"), (pp, "pp"), (pp, "pp"),
                (pp, "pp"), (ppd, "ppd"), (ppd, "ppd"), (ppd, "ppd"),
            ]
            y_ps = [
                pool.tile([P, NC], f32, name=f"pyf{m}", tag=tag)
                for m, (pool, tag) in enumerate(pools)
            ]
            for j in range(KO):
                for m in range(MT):
                    nc.tensor.matmul(
                        y_ps[m][:, :],
                        wo_sb[:, j * D + m * P: j * D + (m + 1) * P],
                        gated_tiles[c][j][:, :],
                        start=(j == 0),
                        stop=(j == KO - 1),
                    )
            for m in range(MT):
                y_sb = ypool.tile([P, NC], f32, name="y_sb", tag="ysb")
                nc.vector.tensor_copy(y_sb[:, :], y_ps[m][:, :])
                nc.sync.dma_start(yt_m[m, :, csl], y_sb[:, :])
            gated_tiles[c] = [None] * KO

        for c in range(SC):
            x_cur = x_next
            if c + 1 < SC:
                x_next = load_x(c + 1)
            emit_front(c, x_cur, ba_order=(c == SC - 1))
            if c > 0:
                emit_back(c - 1)
        emit_back_final(SC - 1)

    nc.compile()
    return nc


def _get_program():
    if "nc" not in _CACHE:
        _CACHE["nc"] = _build_program()
    return _CACHE["nc"]


def _make_in_maps(x, W_in, W_select, W_gate, W_out, W_delta, log_a):
    a = (1.0 / (1.0 + np.exp(-log_a.astype(np.float32)))).astype(np.float32)
    in_maps = []
    for c in range(N_CORES):
        b, h = divmod(c, 2)
        sl = slice(h * E, (h + 1) * E)
        xT = np.ascontiguousarray(x[b].T)                       # [D, S]
        w4 = np.ascontiguousarray(
            np.stack(
                [
                    W_delta[sl, :].T,
                    0.5 * W_select[sl, :].T,   # sigmoid via tanh(z/2)
                    W_in[sl, :].T,
                    0.5 * W_gate[sl, :].T,     # sigmoid via tanh(z/2)
                ]
            )
        )                                                       # [4, D, E]
        wo = np.ascontiguousarray(0.25 * W_out[:, sl].T)        # [E, D]
        na_m = np.ascontiguousarray((-a[sl]).reshape(ET, P).T)  # [P, ET]
        in_maps.append({"xt": xT, "w4": w4, "wo": wo, "na": na_m})
    return in_maps


def kernel(x, W_in, W_select, W_gate, W_out, W_delta, log_a):
    from concourse.bass_utils import run_bass_kernel_spmd

    nc = _get_program()
    in_maps = _make_in_maps(
        np.asarray(x, np.float32),
        np.asarray(W_in, np.float32),
        np.asarray(W_select, np.float32),
        np.asarray(W_gate, np.float32),
        np.asarray(W_out, np.float32),
        np.asarray(W_delta, np.float32),
        np.asarray(log_a, np.float32),
    )
    res = run_bass_kernel_spmd(nc, in_maps, core_ids=list(range(N_CORES)))
    y = np.empty((B, S, D), np.float32)
    for b in range(B):
        yT = res.results[2 * b]["yt"] + res.results[2 * b + 1]["yt"]
        y[b] = yT.T
    return y


if __name__ == "__main__":
    nc = _get_program()
    print("program built OK")


# revision 28
# speedup vs baseline: 1.0452x; 1.0452x over previous
"""ChaosSSMCore (diag mode) Trainium2 kernel.

Problem: B=4, S=4096, D=1024, fp32.
    delta  = softplus(x @ Wd.T); decay = exp(-delta * sigmoid(log_a))
    update = sigmoid(x @ Ws.T) * tanh(x @ Wi.T)
    gate   = sigmoid(x @ Wg.T)
    state_t = decay_t * state_{t-1} + update_t        (scan over S, elementwise in D)
    y      = (gate * states) @ Wo.T

Sharding: 8 cores = 4 batches x 2 halves of the D output dim. Each core
computes its 512-channel slice of the 4 input projections in [e, s] layout
(channels on partitions, sequence on the free axis), applies activations on
the scalar engine, runs the hardware tensor_tensor_scan (state = d*s + u along
the free dim) on the vector engine, gates, and computes a partial output GEMM
against its 512 columns of W_out. Host sums the two partials per batch and
transposes back.

ACT table sets on trn2 don't contain Softplus, and Exp/Ln live in a different
set than Tanh, so:
  softplus(z) = ln(exp(z) + 1)              (Exp + Ln, one table set)
  sigmoid(z)  = (1 + tanh(z/2)) / 2         (Tanh set; the 1/2 input scale is
                                             folded into W_select/W_gate and
                                             the two 1/2 output scales into
                                             W_out, which becomes W_out/4)
Per sequence chunk all Exp/Ln work runs before all Tanh work, and the ACT
instruction order is pinned with explicit dep edges so the scheduler cannot
interleave phases across chunks: 2 table loads per chunk.

Matmuls run in float32r (full-rate 4-byte matmul path).
"""

import sys

if "/opt/trn_rl_repo" not in sys.path:
    sys.path.insert(0, "/opt/trn_rl_repo")

import numpy as np

# Problem constants (hardcoded per harness contract).
B, S, D = 4, 4096, 1024
P = 128           # SBUF partitions
E = D // 2        # per-core channel slice
NC = 512          # sequence chunk (= fp32 moving-operand max = one PSUM bank)
KT = D // P       # k-tiles per input projection contraction (8)
ET = E // P       # e-tiles per core (4)
KO = E // P       # k'-tiles for the output GEMM contraction (4)
MT = D // P       # output-row tiles (8)
SC = S // NC      # sequence chunks (8)
N_CORES = 8

_CACHE = {}


def _build_program():
    import concourse.bacc as bacc
    import concourse.mybir as mybir
    import concourse.tile as tile
    from concourse.tile import add_dep_helper
    from contextlib import ExitStack

    f32 = mybir.dt.float32
    f32r = mybir.dt.float32r
    AF = mybir.ActivationFunctionType
    OP = mybir.AluOpType

    # The act-table-load inserter picks the FIRST table set containing each
    # activation func. Ln's first set (natural_log) lacks Exp and Exp's
    # (exp_and_friends after natural_log_exp) lacks Ln, so an Exp-Ln-Exp
    # chain reloads tables on every instruction (~2.7us each). Removing the
    # two funcs from their single-func sets (set membership only — dict
    # order, and therefore act_func_set_id numbering, is unchanged) makes
    # both resolve to natural_log_exp_and_others.
    from concourse.hw_specs import get_activation_tables

    tabs = get_activation_tables("gen3")
    tabs["natural_log"].discard(mybir.ActivationFunctionType.Ln)
    tabs["exp_and_friends"].discard(mybir.ActivationFunctionType.Exp)

    nc = bacc.Bacc("TRN2", target_bir_lowering=False)

    xt = nc.declare_dram_parameter("xt", [D, S], f32r, isOutput=False)
    w4 = nc.declare_dram_parameter("w4", [4, D, E], f32r, isOutput=False)
    wo = nc.declare_dram_parameter("wo", [E, D], f32r, isOutput=False)
    na = nc.declare_dram_parameter("na", [P, ET], f32, isOutput=False)
    yt = nc.declare_dram_parameter("yt", [D, S], f32, isOutput=True)

    xt_p = xt.rearrange("(k p) s -> p k s", p=P)        # [P, KT, S]
    w4_p = w4.rearrange("q (k p) e -> q p k e", p=P)    # [4, P, KT, E]
    wo_p = wo.rearrange("(k p) d -> p k d", p=P)        # [P, KO, D]
    yt_m = yt.rearrange("(m p) s -> m p s", p=P)        # [MT, P, S]

    with tile.TileContext(nc) as tc, ExitStack() as ctx:
        wpool = ctx.enter_context(tc.tile_pool(name="w", bufs=1))
        xpool = ctx.enter_context(tc.tile_pool(name="x", bufs=2))
        ppd = ctx.enter_context(tc.tile_pool(name="ppd", bufs=3, space="PSUM"))
        pp = ctx.enter_context(tc.tile_pool(name="pp", bufs=3, space="PSUM"))
        pyp = ctx.enter_context(tc.tile_pool(name="pyp", bufs=2, space="PSUM"))
        dpool = ctx.enter_context(tc.tile_pool(name="dp", bufs=2))
        decpool = ctx.enter_context(tc.tile_pool(name="dec", bufs=6))
        spool = ctx.enter_context(tc.tile_pool(name="sp", bufs=4))
        tpool = ctx.enter_context(tc.tile_pool(name="tp", bufs=4))
        upool = ctx.enter_context(tc.tile_pool(name="up", bufs=2))
        stpool = ctx.enter_context(tc.tile_pool(name="stp", bufs=6))
        gpool = ctx.enter_context(tc.tile_pool(name="gp", bufs=4))
        gdpool = ctx.enter_context(tc.tile_pool(name="gdp", bufs=6))
        ypool = ctx.enter_context(tc.tile_pool(name="yp", bufs=4))

        # Pin the ACT instruction order to the emission order so the
        # scheduler can't interleave Exp/Ln and Tanh phases (each crossing
        # costs a ~2.7us ACT table load).
        last_act = [None]

        def act(*args, **kwargs):
            h = nc.scalar.activation(*args, **kwargs)
            if last_act[0] is not None:
                add_dep_helper(h.ins, last_act[0].ins, sync=False,
                               reason="pin ACT table phase order")
            last_act[0] = h
            return h

        def load_w(q, split=False):
            wq = wpool.tile([P, KT * E], f32r, name=f"w{q}_sb", tag=f"w{q}")
            if split:
                # Per-k-tile DMAs: the first matmul only waits for its own
                # k-slice instead of the full 2MB tensor.
                for k in range(KT):
                    nc.sync.dma_start(
                        wq[:, k * E:(k + 1) * E], w4_p[q, :, k, :]
                    )
            else:
                nc.sync.dma_start(wq.rearrange("p (k e) -> p k e", k=KT), w4_p[q])
            return wq

        def load_x(c):
            x_sb = xpool.tile([P, KT * NC], f32r, name="x_sb", tag="x")
            for k in range(KT):
                nc.sync.dma_start(
                    x_sb[:, k * NC:(k + 1) * NC],
                    xt_p[:, k, c * NC:(c + 1) * NC],
                )
            return x_sb

        # Startup order: interleave the delta-weight and first-x k-slices so
        # the first matmul starts after ~2 small DMAs, then stream the three
        # gate weights. wo/na are not needed until the first emit_back
        # (~90us in) and load last.
        w_sb = [None] * 4
        w_sb[0] = wpool.tile([P, KT * E], f32r, name="w0_sb", tag="w0")
        x_next = xpool.tile([P, KT * NC], f32r, name="x_sb", tag="x")
        na_sb = wpool.tile([P, ET], f32, name="na_sb", tag="na")
        for k in range(KT):
            nc.sync.dma_start(
                w_sb[0][:, k * E:(k + 1) * E], w4_p[0, :, k, :]
            )
            nc.sync.dma_start(
                x_next[:, k * NC:(k + 1) * NC], xt_p[:, k, 0:NC]
            )
        nc.sync.dma_start(na_sb[:, :], na[:, :])
        for q in (1, 2, 3):
            w_sb[q] = load_w(q, split=True)
        wo_sb = wpool.tile([P, KO * D], f32r, name="wo_sb", tag="wo")
        nc.sync.dma_start(wo_sb.rearrange("p (k d) -> p k d", k=KO), wo_p)

        prev_states = [None] * ET
        gated_tiles = [[None] * ET for _ in range(SC)]

        def mm_group(ps, q, j, x_sb):
            for k in range(KT):
                nc.tensor.matmul(
                    ps[:, :],
                    w_sb[q][:, k * E + j * P: k * E + (j + 1) * P],
                    x_sb[:, k * NC:(k + 1) * NC],
                    start=(k == 0),
                    stop=(k == KT - 1),
                )

        def phase_a(c, x_sb):
            # Decay path (Exp/Ln table). All four Exps run first so every
            # delta psum is drained before the Ln table-load (~2.7us) stalls
            # the ACT engine; decay lands in SBUF.
            dec_t = []
            e1_t = []
            for j in range(ET):
                ps = ppd.tile([P, NC], f32, name="psd", tag="ppd")
                mm_group(ps, 0, j, x_sb)
                e1 = dpool.tile([P, NC], f32, name="e1", tag="e1", bufs=5)
                act(e1[:, :], ps[:, :], AF.Exp)
                e1_t.append(e1)
            for j in range(ET):
                l1 = dpool.tile([P, NC], f32, name="l1", tag="l1")
                act(l1[:, :], e1_t[j][:, :], AF.Ln, bias=1.0)
                d = decpool.tile([P, NC], f32, name="dec", tag="dec")
                act(d[:, :], l1[:, :], AF.Exp, scale=na_sb[:, j:j + 1])
                dec_t.append(d)
            return dec_t

        def phase_b(c, x_sb):
            # The three gate projections (Tanh table).
            tS, tI, tG = [], [], []
            for j in range(ET):
                for q, lst, pool, nm in (
                    (1, tS, spool, "tS"),
                    (2, tI, tpool, "tI"),
                    (3, tG, gpool, "tG"),
                ):
                    ps = pp.tile([P, NC], f32, name="ps", tag="pp")
                    mm_group(ps, q, j, x_sb)
                    t = pool.tile([P, NC], f32, name=nm, tag=nm)
                    act(t[:, :], ps[:, :], AF.Tanh)
                    lst.append(t)
            return tS, tI, tG

        def emit_front(c, x_sb, ba_order=False):
            # For the last chunk run phase B before phase A: the Tanh work
            # overlaps the remaining PE matmuls, the final scan chain starts
            # earlier, and B(last) reuses B(prev)'s table (one load saved).
            if ba_order:
                tS, tI, tG = phase_b(c, x_sb)
                dec_t = phase_a(c, x_sb)
            else:
                dec_t = phase_a(c, x_sb)
                tS, tI, tG = phase_b(c, x_sb)

            # Phase C: update' = (1+tS)*tI ; scan ; gated' = (1+tG)*st.
            for j in range(ET):
                u = upool.tile([P, NC], f32, name="upd", tag="upd")
                nc.vector.scalar_tensor_tensor(
                    u[:, :], tS[j][:, :], 1.0, tI[j][:, :],
                    op0=OP.add, op1=OP.mult,
                )
                st = stpool.tile([P, NC], f32, name="st", tag="st")
                init = 0.0 if c == 0 else prev_states[j][:, NC - 1:NC]
                nc.vector.tensor_tensor_scan(
                    st[:, :], dec_t[j][:, :], u[:, :], init,
                    op0=OP.mult, op1=OP.add,
                )
                prev_states[j] = st
                g = gdpool.tile([P, NC], f32r, name="gated", tag="gated")
                nc.vector.scalar_tensor_tensor(
                    g[:, :], tG[j][:, :], 1.0, st[:, :],
                    op0=OP.add, op1=OP.mult,
                )
                gated_tiles[c][j] = g

        def emit_back(c):
            csl = slice(c * NC, (c + 1) * NC)
            for m in range(MT):
                py = pyp.tile([P, NC], f32, name="py", tag="py")
                for j in range(KO):
                    nc.tensor.matmul(
                        py[:, :],
                        wo_sb[:, j * D + m * P: j * D + (m + 1) * P],
                        gated_tiles[c][j][:, :],
                        start=(j == 0),
                        stop=(j == KO - 1),
                    )
                y_sb = ypool.tile([P, NC], f32, name="y_sb", tag="ysb")
                nc.vector.tensor_copy(y_sb[:, :], py[:, :])
                nc.sync.dma_start(yt_m[m, :, csl], y_sb[:, :])
            gated_tiles[c] = [None] * KO

        def emit_back_final(c):
            # Last chunk: no more projection matmuls will run, so all 8 PSUM
            # banks are free. Accumulate j-major into 8 live psum tiles so
            # the PE streams 8 matmuls per gated tile as soon as it lands
            # instead of stalling on each j four times.
            csl = slice(c * NC, (c + 1) * NC)
            pools = [
                (pyp, "py"), (pyp, "py"), (pp, "pp"), (pp, "pp"),
                (pp, "pp"), (ppd, "ppd"), (ppd, "ppd"), (ppd, "ppd"),
            ]
            y_ps = [
                pool.tile([P, NC], f32, name=f"pyf{m}", tag=tag)
                for m, (pool, tag) in enumerate(pools)
            ]
            for j in range(KO):
                for m in range(MT):
                    nc.tensor.matmul(
                        y_ps[m][:, :],
                        wo_sb[:, j * D + m * P: j * D + (m + 1) * P],
                        gated_tiles[c][j][:, :],
                        start=(j == 0),
                        stop=(j == KO - 1),
                    )
            for m in range(MT):
                y_sb = ypool.tile([P, NC], f32, name="y_sb", tag="ysb")
                nc.vector.tensor_copy(y_sb[:, :], y_ps[m][:, :])
                nc.sync.dma_start(yt_m[m, :, csl], y_sb[:, :])
            gated_tiles[c] = [None] * KO

        for c in range(SC):
            x_cur = x_next
            if c + 1 < SC:
                x_next = load_x(c + 1)
            emit_front(c, x_cur)
            if c > 0:
                emit_back(c - 1)
        emit_back_final(SC - 1)

    nc.compile()
    return nc


def _get_program():
    if "nc" not in _CACHE:
        _CACHE["nc"] = _build_program()
    return _CACHE["nc"]


def _make_in_maps(x, W_in, W_select, W_gate, W_out, W_delta, log_a):
    a = (1.0 / (1.0 + np.exp(-log_a.astype(np.float32)))).astype(np.float32)
    in_maps = []
    for c in range(N_CORES):
        b, h = divmod(c, 2)
        sl = slice(h * E, (h + 1) * E)
        xT = np.ascontiguousarray(x[b].T)                       # [D, S]
        w4 = np.ascontiguousarray(
            np.stack(
                [
                    W_delta[sl, :].T,
                    0.5 * W_select[sl, :].T,   # sigmoid via tanh(z/2)
                    W_in[sl, :].T,
                    0.5 * W_gate[sl, :].T,     # sigmoid via tanh(z/2)
                ]
            )
        )                                                       # [4, D, E]
        wo = np.ascontiguousarray(0.25 * W_out[:, sl].T)        # [E, D]
        na_m = np.ascontiguousarray((-a[sl]).reshape(ET, P).T)  # [P, ET]
        in_maps.append({"xt": xT, "w4": w4, "wo": wo, "na": na_m})
    return in_maps


def kernel(x, W_in, W_select, W_gate, W_out, W_delta, log_a):
    from concourse.bass_utils import run_bass_kernel_spmd

    nc = _get_program()
    in_maps = _make_in_maps(
        np.asarray(x, np.float32),
        np.asarray(W_in, np.float32),
        np.asarray(W_select, np.float32),
        np.asarray(W_gate, np.float32),
        np.asarray(W_out, np.float32),
        np.asarray(W_delta, np.float32),
        np.asarray(log_a, np.float32),
    )
    res = run_bass_kernel_spmd(nc, in_maps, core_ids=list(range(N_CORES)))
    y = np.empty((B, S, D), np.float32)
    for b in range(B):
        yT = res.results[2 * b]["yt"] + res.results[2 * b + 1]["yt"]
        y[b] = yT.T
    return y


if __name__ == "__main__":
    nc = _get_program()
    print("program built OK")


# revision 29
# speedup vs baseline: 1.0454x; 1.0002x over previous
"""ChaosSSMCore (diag mode) Trainium2 kernel.

Problem: B=4, S=4096, D=1024, fp32.
    delta  = softplus(x @ Wd.T); decay = exp(-delta * sigmoid(log_a))
    update = sigmoid(x @ Ws.T) * tanh(x @ Wi.T)
    gate   = sigmoid(x @ Wg.T)
    state_t = decay_t * state_{t-1} + update_t        (scan over S, elementwise in D)
    y      = (gate * states) @ Wo.T

Sharding: 8 cores = 4 batches x 2 halves of the D output dim. Each core
computes its 512-channel slice of the 4 input projections in [e, s] layout
(channels on partitions, sequence on the free axis), applies activations on
the scalar engine, runs the hardware tensor_tensor_scan (state = d*s + u along
the free dim) on the vector engine, gates, and computes a partial output GEMM
against its 512 columns of W_out. Host sums the two partials per batch and
transposes back.

ACT table sets on trn2 don't contain Softplus, and Exp/Ln live in a different
set than Tanh, so:
  softplus(z) = ln(exp(z) + 1)              (Exp + Ln, one table set)
  sigmoid(z)  = (1 + tanh(z/2)) / 2         (Tanh set; the 1/2 input scale is
                                             folded into W_select/W_gate and
                                             the two 1/2 output scales into
                                             W_out, which becomes W_out/4)
Per sequence chunk all Exp/Ln work runs before all Tanh work, and the ACT
instruction order is pinned with explicit dep edges so the scheduler cannot
interleave phases across chunks: 2 table loads per chunk.

Matmuls run in float32r (full-rate 4-byte matmul path).
"""

import sys

if "/opt/trn_rl_repo" not in sys.path:
    sys.path.insert(0, "/opt/trn_rl_repo")

import numpy as np

# Problem constants (hardcoded per harness contract).
B, S, D = 4, 4096, 1024
P = 128           # SBUF partitions
E = D // 2        # per-core channel slice
NC = 512          # sequence chunk (= fp32 moving-operand max = one PSUM bank)
KT = D // P       # k-tiles per input projection contraction (8)
ET = E // P       # e-tiles per core (4)
KO = E // P       # k'-tiles for the output GEMM contraction (4)
MT = D // P       # output-row tiles (8)
SC = S // NC      # sequence chunks (8)
N_CORES = 8

_CACHE = {}


def _build_program():
    import concourse.bacc as bacc
    import concourse.mybir as mybir
    import concourse.tile as tile
    from concourse.tile import add_dep_helper
    from contextlib import ExitStack

    f32 = mybir.dt.float32
    f32r = mybir.dt.float32r
    AF = mybir.ActivationFunctionType
    OP = mybir.AluOpType

    # The act-table-load inserter picks the FIRST table set containing each
    # activation func. Ln's first set (natural_log) lacks Exp and Exp's
    # (exp_and_friends after natural_log_exp) lacks Ln, so an Exp-Ln-Exp
    # chain reloads tables on every instruction (~2.7us each). Removing the
    # two funcs from their single-func sets (set membership only — dict
    # order, and therefore act_func_set_id numbering, is unchanged) makes
    # both resolve to natural_log_exp_and_others.
    from concourse.hw_specs import get_activation_tables

    tabs = get_activation_tables("gen3")
    tabs["natural_log"].discard(mybir.ActivationFunctionType.Ln)
    tabs["exp_and_friends"].discard(mybir.ActivationFunctionType.Exp)

    nc = bacc.Bacc("TRN2", target_bir_lowering=False)

    xt = nc.declare_dram_parameter("xt", [D, S], f32r, isOutput=False)
    w4 = nc.declare_dram_parameter("w4", [4, D, E], f32r, isOutput=False)
    wo = nc.declare_dram_parameter("wo", [E, D], f32r, isOutput=False)
    na = nc.declare_dram_parameter("na", [P, ET], f32, isOutput=False)
    yt = nc.declare_dram_parameter("yt", [D, S], f32, isOutput=True)

    xt_p = xt.rearrange("(k p) s -> p k s", p=P)        # [P, KT, S]
    w4_p = w4.rearrange("q (k p) e -> q p k e", p=P)    # [4, P, KT, E]
    wo_p = wo.rearrange("(k p) d -> p k d", p=P)        # [P, KO, D]
    yt_m = yt.rearrange("(m p) s -> m p s", p=P)        # [MT, P, S]

    with tile.TileContext(nc) as tc, ExitStack() as ctx:
        wpool = ctx.enter_context(tc.tile_pool(name="w", bufs=1))
        xpool = ctx.enter_context(tc.tile_pool(name="x", bufs=2))
        ppd = ctx.enter_context(tc.tile_pool(name="ppd", bufs=3, space="PSUM"))
        pp = ctx.enter_context(tc.tile_pool(name="pp", bufs=3, space="PSUM"))
        pyp = ctx.enter_context(tc.tile_pool(name="pyp", bufs=2, space="PSUM"))
        dpool = ctx.enter_context(tc.tile_pool(name="dp", bufs=2))
        decpool = ctx.enter_context(tc.tile_pool(name="dec", bufs=6))
        spool = ctx.enter_context(tc.tile_pool(name="sp", bufs=4))
        tpool = ctx.enter_context(tc.tile_pool(name="tp", bufs=4))
        upool = ctx.enter_context(tc.tile_pool(name="up", bufs=2))
        stpool = ctx.enter_context(tc.tile_pool(name="stp", bufs=6))
        gpool = ctx.enter_context(tc.tile_pool(name="gp", bufs=4))
        gdpool = ctx.enter_context(tc.tile_pool(name="gdp", bufs=6))
        ypool = ctx.enter_context(tc.tile_pool(name="yp", bufs=4))

        # Pin the ACT instruction order to the emission order so the
        # scheduler can't interleave Exp/Ln and Tanh phases (each crossing
        # costs a ~2.7us ACT table load).
        last_act = [None]

        def act(*args, **kwargs):
            h = nc.scalar.activation(*args, **kwargs)
            if last_act[0] is not None:
                add_dep_helper(h.ins, last_act[0].ins, sync=False,
                               reason="pin ACT table phase order")
            last_act[0] = h
            return h

        def load_w(q, split=False):
            wq = wpool.tile([P, KT * E], f32r, name=f"w{q}_sb", tag=f"w{q}")
            if split:
                # Per-k-tile DMAs: the first matmul only waits for its own
                # k-slice instead of the full 2MB tensor.
                for k in range(KT):
                    nc.sync.dma_start(
                        wq[:, k * E:(k + 1) * E], w4_p[q, :, k, :]
                    )
            else:
                nc.sync.dma_start(wq.rearrange("p (k e) -> p k e", k=KT), w4_p[q])
            return wq

        def load_x(c):
            x_sb = xpool.tile([P, KT * NC], f32r, name="x_sb", tag="x")
            for k in range(KT):
                nc.sync.dma_start(
                    x_sb[:, k * NC:(k + 1) * NC],
                    xt_p[:, k, c * NC:(c + 1) * NC],
                )
            return x_sb

        # Startup order: interleave the delta-weight and first-x k-slices so
        # the first matmul starts after ~2 small DMAs, then stream the three
        # gate weights. wo/na are not needed until the first emit_back
        # (~90us in) and load last.
        w_sb = [None] * 4
        w_sb[0] = wpool.tile([P, KT * E], f32r, name="w0_sb", tag="w0")
        x_next = xpool.tile([P, KT * NC], f32r, name="x_sb", tag="x")
        na_sb = wpool.tile([P, ET], f32, name="na_sb", tag="na")
        for k in range(KT):
            nc.sync.dma_start(
                w_sb[0][:, k * E:(k + 1) * E], w4_p[0, :, k, :]
            )
            nc.sync.dma_start(
                x_next[:, k * NC:(k + 1) * NC], xt_p[:, k, 0:NC]
            )
        nc.sync.dma_start(na_sb[:, :], na[:, :])
        for q in (1, 2, 3):
            w_sb[q] = load_w(q, split=True)
        wo_sb = wpool.tile([P, KO * D], f32r, name="wo_sb", tag="wo")
        nc.sync.dma_start(wo_sb.rearrange("p (k d) -> p k d", k=KO), wo_p)

        prev_states = [None] * ET
        gated_tiles = [[None] * ET for _ in range(SC)]

        def mm_group(ps, q, j, x_sb):
            for k in range(KT):
                nc.tensor.matmul(
                    ps[:, :],
                    w_sb[q][:, k * E + j * P: k * E + (j + 1) * P],
                    x_sb[:, k * NC:(k + 1) * NC],
                    start=(k == 0),
                    stop=(k == KT - 1),
                )

        def phase_a(c, x_sb):
            # Decay path (Exp/Ln table). All four Exps run first so every
            # delta psum is drained before the Ln table-load (~2.7us) stalls
            # the ACT engine; decay lands in SBUF.
            dec_t = []
            e1_t = []
            for j in range(ET):
                ps = ppd.tile([P, NC], f32, name="psd", tag="ppd")
                mm_group(ps, 0, j, x_sb)
                e1 = dpool.tile([P, NC], f32, name="e1", tag="e1", bufs=5)
                act(e1[:, :], ps[:, :], AF.Exp)
                e1_t.append(e1)
            for j in range(ET):
                l1 = dpool.tile([P, NC], f32, name="l1", tag="l1")
                act(l1[:, :], e1_t[j][:, :], AF.Ln, bias=1.0)
                d = decpool.tile([P, NC], f32, name="dec", tag="dec")
                act(d[:, :], l1[:, :], AF.Exp, scale=na_sb[:, j:j + 1])
                dec_t.append(d)
            return dec_t

        def phase_b(c, x_sb):
            # The three gate projections (Tanh table).
            tS, tI, tG = [], [], []
            for j in range(ET):
                for q, lst, pool, nm in (
                    (1, tS, spool, "tS"),
                    (2, tI, tpool, "tI"),
                    (3, tG, gpool, "tG"),
                ):
                    ps = pp.tile([P, NC], f32, name="ps", tag="pp")
                    mm_group(ps, q, j, x_sb)
                    t = pool.tile([P, NC], f32, name=nm, tag=nm)
                    act(t[:, :], ps[:, :], AF.Tanh)
                    lst.append(t)
            return tS, tI, tG

        def emit_front(c, x_sb, ba_order=False):
            # For the last chunk run phase B before phase A: the Tanh work
            # overlaps the remaining PE matmuls, the final scan chain starts
            # earlier, and B(last) reuses B(prev)'s table (one load saved).
            if ba_order:
                tS, tI, tG = phase_b(c, x_sb)
                dec_t = phase_a(c, x_sb)
            else:
                dec_t = phase_a(c, x_sb)
                tS, tI, tG = phase_b(c, x_sb)

            # Phase C: update' = (1+tS)*tI ; scan ; gated' = (1+tG)*st.
            for j in range(ET):
                u = upool.tile([P, NC], f32, name="upd", tag="upd")
                nc.vector.scalar_tensor_tensor(
                    u[:, :], tS[j][:, :], 1.0, tI[j][:, :],
                    op0=OP.add, op1=OP.mult,
                )
                st = stpool.tile([P, NC], f32, name="st", tag="st")
                init = 0.0 if c == 0 else prev_states[j][:, NC - 1:NC]
                nc.vector.tensor_tensor_scan(
                    st[:, :], dec_t[j][:, :], u[:, :], init,
                    op0=OP.mult, op1=OP.add,
                )
                prev_states[j] = st
                g = gdpool.tile([P, NC], f32r, name="gated", tag="gated")
                nc.vector.scalar_tensor_tensor(
                    g[:, :], tG[j][:, :], 1.0, st[:, :],
                    op0=OP.add, op1=OP.mult,
                )
                gated_tiles[c][j] = g

        def emit_back(c):
            csl = slice(c * NC, (c + 1) * NC)
            for m in range(MT):
                py = pyp.tile([P, NC], f32, name="py", tag="py")
                for j in range(KO):
                    nc.tensor.matmul(
                        py[:, :],
                        wo_sb[:, j * D + m * P: j * D + (m + 1) * P],
                        gated_tiles[c][j][:, :],
                        start=(j == 0),
                        stop=(j == KO - 1),
                    )
                y_sb = ypool.tile([P, NC], f32, name="y_sb", tag="ysb")
                nc.vector.tensor_copy(y_sb[:, :], py[:, :])
                nc.sync.dma_start(yt_m[m, :, csl], y_sb[:, :])
            gated_tiles[c] = [None] * KO

        def emit_back_final(c):
            # Last chunk: no more projection matmuls will run, so all 8 PSUM
            # banks are free. Accumulate j-major into 8 live psum tiles so
            # the PE streams 8 matmuls per gated tile as soon as it lands
            # instead of stalling on each j four times.
            csl = slice(c * NC, (c + 1) * NC)
            pools = [
                (pyp, "py"), (pyp, "py"), (pp, "pp"), (pp, "pp"),
                (pp, "pp"), (ppd, "ppd"), (ppd, "ppd"), (ppd, "ppd"),
            ]
            y_ps = [
                pool.tile([P, NC], f32, name=f"pyf{m}", tag=tag)
                for m, (pool, tag) in enumerate(pools)
            ]
            for j in range(KO - 1):
                for m in range(MT):
                    nc.tensor.matmul(
                        y_ps[m][:, :],
                        wo_sb[:, j * D + m * P: j * D + (m + 1) * P],
                        gated_tiles[c][j][:, :],
                        start=(j == 0),
                        stop=False,
                    )
            # Final contraction step per m with the copy+store immediately
            # behind it, so the drain streams during the last matmul round
            # instead of bunching after it.
            j = KO - 1
            for m in range(MT):
                nc.tensor.matmul(
                    y_ps[m][:, :],
                    wo_sb[:, j * D + m * P: j * D + (m + 1) * P],
                    gated_tiles[c][j][:, :],
                    start=False,
                    stop=True,
                )
                y_sb = ypool.tile([P, NC], f32, name="y_sb", tag="ysb")
                nc.vector.tensor_copy(y_sb[:, :], y_ps[m][:, :])
                nc.sync.dma_start(yt_m[m, :, csl], y_sb[:, :])
            gated_tiles[c] = [None] * KO

        for c in range(SC):
            x_cur = x_next
            if c + 1 < SC:
                x_next = load_x(c + 1)
            emit_front(c, x_cur)
            if c > 0:
                emit_back(c - 1)
        emit_back_final(SC - 1)

    nc.compile()
    return nc


def _get_program():
    if "nc" not in _CACHE:
        _CACHE["nc"] = _build_program()
    return _CACHE["nc"]


def _make_in_maps(x, W_in, W_select, W_gate, W_out, W_delta, log_a):
    a = (1.0 / (1.0 + np.exp(-log_a.astype(np.float32)))).astype(np.float32)
    in_maps = []
    for c in range(N_CORES):
        b, h = divmod(c, 2)
        sl = slice(h * E, (h + 1) * E)
        xT = np.ascontiguousarray(x[b].T)                       # [D, S]
        w4 = np.ascontiguousarray(
            np.stack(
                [
                    W_delta[sl, :].T,
                    0.5 * W_select[sl, :].T,   # sigmoid via tanh(z/2)
                    W_in[sl, :].T,
                    0.5 * W_gate[sl, :].T,     # sigmoid via tanh(z/2)
                ]
            )
        )                                                       # [4, D, E]
        wo = np.ascontiguousarray(0.25 * W_out[:, sl].T)        # [E, D]
        na_m = np.ascontiguousarray((-a[sl]).reshape(ET, P).T)  # [P, ET]
        in_maps.append({"xt": xT, "w4": w4, "wo": wo, "na": na_m})
    return in_maps


def kernel(x, W_in, W_select, W_gate, W_out, W_delta, log_a):
    from concourse.bass_utils import run_bass_kernel_spmd

    nc = _get_program()
    in_maps = _make_in_maps(
        np.asarray(x, np.float32),
        np.asarray(W_in, np.float32),
        np.asarray(W_select, np.float32),
        np.asarray(W_gate, np.float32),
        np.asarray(W_out, np.float32),
        np.asarray(W_delta, np.float32),
        np.asarray(log_a, np.float32),
    )
    res = run_bass_kernel_spmd(nc, in_maps, core_ids=list(range(N_CORES)))
    y = np.empty((B, S, D), np.float32)
    for b in range(B):
        yT = res.results[2 * b]["yt"] + res.results[2 * b + 1]["yt"]
        y[b] = yT.T
    return y


if __name__ == "__main__":
    nc = _get_program()
    print("program built OK")
